# revision 9
# baseline (speedup 1.0000x reference)
"""Self-contained Trainium2 kernel for nn_Linear_14293651161742.

Computes y[m,o] = sum_k x[m,k] * weight[o,k] * w_scale[o//128, k//128]
(the reference's act_quant divide/multiply round-trip is an exact no-op up
to fp32 rounding, far below the matmul noise floor).

Strategy: shard M across the 8 cores (each core reads the full weight once
plus its x slice -- less HBM traffic than the column-parallel hint, which
replicates the much larger x). All scale folding, transposition, and dtype
casts happen on the host; the device runs a pure GEMM stream.

Precision/speed split: the PE runs fp8(e4m3) matmuls at 2x bf16 throughput
via MatmulPerfMode.DoubleRow, but e4m3's 3 mantissa bits give a ~3.7e-2
relative GEMM error -- over the 2e-2 budget. So the K=4096 contraction is
split: 24 of the 32 128-wide k-tiles run in bf16 and 8 run as 4 fp8
DoubleRow pair-tiles (256-deep contraction each, 1 cycle per output
column). Measured end-to-end relative error 1.9e-2; PE cycle count drops
to 28/32 of the all-bf16 kernel.

Schedule: one o-tile (128 outputs) per round; w slabs stream 4 rounds
ahead on the sync queue while x (fully SBUF-resident) streams on the
vector queue. Round 0 covers three o-tiles so the PE has enough work per
arriving x k-tile to never stall while x streams in; the first three w
slabs are loaded in interleaved k-chunks so all three o-tiles' early
k-tiles land first.
"""

import sys

if "/opt/trn_rl_repo" not in sys.path:
    sys.path.insert(0, "/opt/trn_rl_repo")

import ml_dtypes
import numpy as np

import concourse.bacc as bacc
import concourse.mybir as mybir
import concourse.tile as tile
from concourse import bass_utils

P = 128
N_CORES = 8
KF_TILES = 8          # k-tiles computed in fp8 (must be even)

F32 = mybir.dt.float32
BF16 = mybir.dt.bfloat16
FP8 = mybir.dt.float8e4

NP_BF16 = ml_dtypes.bfloat16
NP_FP8 = ml_dtypes.float8_e4m3fn


def build_gemm_nc(M_loc: int, K: int, O: int, kf: int):
    """Per-core program: yt[ot, p, m] = sum_k w'[ot*128+p, k] * x[m, k].

    Inputs (per core, all host-prepped):
      xb [KB, P, M_loc]        bf16 : x k-tile kt, partition p = k in tile
      x8 [KF/2, P, 2, M_loc]   fp8  : fp8 k-pair t, slot i -> k tile KB+2t+i
      wb [OB, P, KB*P]         bf16 : per o-tile slab, [p][kt][o] packed
      w8 [OB, P, (KF/2)*2*P]   fp8  : per o-tile slab, [p][t][i][o] packed
    Output:
      yt [OB, P, M_loc]        f32  : y^T slice (host transposes back)
    """
    KT = K // P
    KB = KT - kf           # bf16 k-tiles
    PAIRS = kf // 2        # fp8 DoubleRow pair-tiles
    OB = O // P
    MCW = min(512, M_loc)  # bf16 moving chunk (max 512)
    MC = M_loc // MCW
    DCW = min(512, M_loc)  # fp8 DoubleRow chunk: 512 outputs = 512 moving
                           # pair-rows (the 512-row stream limit counts pairs)
    DC = M_loc // DCW
    HALVES = MCW // DCW    # DR chunks per psum tile
    DR = mybir.MatmulPerfMode.DoubleRow

    nc = bacc.Bacc("TRN2", target_bir_lowering=False, debug=False)
    xb = nc.dram_tensor("xb", [KB, P, M_loc], BF16, kind="ExternalInput")
    wbt = nc.dram_tensor("wb", [OB, P, KB * P], BF16, kind="ExternalInput")
    yt = nc.dram_tensor("yt", [OB, P, M_loc], F32, kind="ExternalOutput")
    if PAIRS:
        x8 = nc.dram_tensor("x8", [PAIRS, P, 2 * M_loc], FP8, kind="ExternalInput")
        w8t = nc.dram_tensor("w8", [OB, P, PAIRS * 2 * P], FP8, kind="ExternalInput")

    # Round 0 covers four o-tiles so the PE has ~1.7us of work per arriving
    # x k-tile (x streams for ~35us) and never stalls on the x stream;
    # later rounds one o-tile each.
    R0 = min(4, OB)
    rounds = [list(range(R0))] + [[ot] for ot in range(R0, OB)]
    W_PREFETCH = 6         # o-tile w slabs in flight (= pool bufs)

    with tile.TileContext(nc) as tc:
        with (
            tc.tile_pool(name="xpool", bufs=1) as x_pool,
            tc.tile_pool(name="wbp", bufs=W_PREFETCH) as wb_pool,
            tc.tile_pool(name="w8p", bufs=W_PREFETCH) as w8_pool,
            tc.tile_pool(name="yout", bufs=4) as y_pool,
            tc.tile_pool(name="psum", bufs=4, space="PSUM") as psum_pool,
        ):
            wb_sb = {}
            w8_sb = {}

            def alloc_w(ot):
                wb_sb[ot] = wb_pool.tile([P, KB, P], BF16, tag="wb",
                                         name=f"wb{ot}")
                if PAIRS:
                    w8_sb[ot] = w8_pool.tile([P, PAIRS, 2, P], FP8, tag="w8",
                                             name=f"w8{ot}")

            def load_wb_chunk(ot, k0, k1, eng=None):
                (eng or nc.sync).dma_start(
                    wb_sb[ot][:, k0:k1].rearrange("p kt o -> p (kt o)"),
                    wbt.ap()[ot, :, k0 * P:k1 * P],
                )

            def load_w8(ot, eng=None):
                if PAIRS:
                    (eng or nc.sync).dma_start(
                        w8_sb[ot][:].rearrange("p t i o -> p (t i o)"),
                        w8t.ap()[ot],
                    )

            # Prologue. The first two x k-tiles ride at the head of the sync
            # queue (earliest to start) with a tiny first w chunk, so the
            # first matmul fires as soon as the PE sequencer boots; the rest
            # of x streams on the scalar queue (HWDGE), with the fp8 x pairs
            # placed mid-stream so they land well before round 0's DR phase.
            xb_sb = [None] * KB
            x8_sb = [None] * PAIRS

            def load_x(kt, eng):
                t = x_pool.tile([P, M_loc], BF16, tag=f"xb{kt}", name=f"xb{kt}")
                eng.dma_start(t[:], xb.ap()[kt])
                xb_sb[kt] = t

            def load_x8(pr, eng):
                t = x_pool.tile([P, 2, M_loc], FP8, tag=f"x8{pr}", name=f"x8{pr}")
                eng.dma_start(t[:].rearrange("p i m -> p (i m)"), x8.ap()[pr])
                x8_sb[pr] = t

            head = min(2, KB)
            for kt in range(head):
                load_x(kt, nc.sync)

            for ot in range(R0):
                alloc_w(ot)
            bounds = [b for b in (0, 2, KB // 3, (2 * KB) // 3, KB)
                      if 0 <= b <= KB]
            bounds = sorted(set(bounds))
            for g in range(len(bounds) - 1):
                for ot in range(R0):
                    load_wb_chunk(ot, bounds[g], bounds[g + 1])
            for ot in range(R0):
                load_w8(ot)

            mid = KB // 2
            for kt in range(head, mid):
                load_x(kt, nc.scalar)
            for pr in range(PAIRS):
                load_x8(pr, nc.scalar)
            for kt in range(mid, KB):
                load_x(kt, nc.scalar)

            # Prefetch two more slabs behind x on the scalar queue so they
            # don't compete with the round-0 x stream for HBM bandwidth.
            next_w = R0
            for _ in range(min(2, OB - next_w)):
                alloc_w(next_w)
                load_wb_chunk(next_w, 0, KB, nc.scalar)
                load_w8(next_w, nc.scalar)
                next_w += 1

            for rnd in rounds:
                last_round = rnd is rounds[-1]
                psums = {}
                for ot in rnd:
                    for mc in range(MC):
                        psums[(ot, mc)] = psum_pool.tile(
                            [P, MCW], F32, tag=f"ps{mc}", name=f"ps{ot}_{mc}"
                        )
                # bf16 k-tiles (start flag on kt 0, full-width writes)
                for kt in range(KB):
                    for ot in rnd:
                        for mc in range(MC):
                            nc.tensor.matmul(
                                psums[(ot, mc)][:],
                                wb_sb[ot][:, kt],
                                xb_sb[kt][:, mc * MCW:(mc + 1) * MCW],
                                start=(kt == 0),
                                stop=(PAIRS == 0 and kt == KB - 1),
                            )
                # fp8 DoubleRow pair-tiles; stop on the last write per psum
                # tile (zero-region) only. In the last round iterate chunks
                # outermost so psum tile 0 completes (and can evict) while
                # the PE still works on tile 1.
                if PAIRS:
                    dr_order = (
                        [(pr, c) for c in range(DC) for pr in range(PAIRS)]
                        if last_round else
                        [(pr, c) for pr in range(PAIRS) for c in range(DC)]
                    )
                    for ot in rnd:
                        for pr, c in dr_order:
                            mc, half = divmod(c, HALVES)
                            nc.tensor.matmul(
                                psums[(ot, mc)][:, half * DCW:(half + 1) * DCW],
                                w8_sb[ot][:, pr],
                                x8_sb[pr][:, :, c * DCW:(c + 1) * DCW],
                                start=False,
                                stop=(pr == PAIRS - 1 and half == HALVES - 1),
                                perf_mode=DR,
                            )
                # evict on DVE, store on the scalar-engine DMA queue (the
                # last round splits the store across two queues per-mc so
                # the tail drains sooner)
                for ot in rnd:
                    ysb = y_pool.tile([P, M_loc], F32, tag="y", name=f"y{ot}")
                    for mc in range(MC):
                        nc.vector.tensor_copy(
                            ysb[:, mc * MCW:(mc + 1) * MCW], psums[(ot, mc)][:]
                        )
                        if last_round:
                            eng = nc.scalar if mc % 2 == 0 else nc.sync
                            eng.dma_start(
                                yt.ap()[ot, :, mc * MCW:(mc + 1) * MCW],
                                ysb[:, mc * MCW:(mc + 1) * MCW],
                            )
                    if not last_round:
                        nc.gpsimd.dma_start(yt.ap()[ot], ysb[:])
                    del wb_sb[ot]
                    if PAIRS:
                        del w8_sb[ot]
                    if next_w < OB:
                        alloc_w(next_w)
                        load_wb_chunk(next_w, 0, KB)
                        load_w8(next_w)
                        next_w += 1
    nc.compile()
    return nc


_CACHED = {}


def _get_nc(M_loc, K, O, kf):
    key = (M_loc, K, O, kf)
    if key not in _CACHED:
        _CACHED[key] = build_gemm_nc(M_loc, K, O, kf)
    return _CACHED[key]


def _prep_weights(weight: np.ndarray, w_scale: np.ndarray, kf: int):
    O, K = weight.shape
    OB, KT = O // P, K // P
    KB = KT - kf
    wdq = (
        weight.reshape(OB, P, KT, P).astype(np.float32)
        * w_scale[:, None, :, None]
    )  # [ot, o, kt, p]
    # bf16 part: [ot, p(k), kt, o] slabs, contiguous per (ot, p)
    wb = np.ascontiguousarray(
        wdq[:, :, :KB].transpose(0, 3, 2, 1)
    ).astype(NP_BF16).reshape(OB, P, KB * P)
    w8 = None
    if kf:
        # fp8 part: [ot, p(k), pair, slot, o]
        w8f = wdq[:, :, KB:].reshape(OB, P, kf // 2, 2, P)  # [ot,o,t,i,p]
        w8 = np.ascontiguousarray(
            w8f.transpose(0, 4, 2, 3, 1)
        ).astype(NP_FP8).reshape(OB, P, (kf // 2) * 2 * P)
    return wb, w8


def kernel(x: np.ndarray, weight: np.ndarray, w_scale: np.ndarray) -> np.ndarray:
    M, K = x.shape
    O = weight.shape[0]
    assert M % N_CORES == 0
    M_loc = M // N_CORES
    kf = KF_TILES
    KT = K // P
    KB = KT - kf

    nc = _get_nc(M_loc, K, O, kf)
    wb, w8 = _prep_weights(weight, w_scale, kf)

    in_maps = []
    for c in range(N_CORES):
        xt_c = np.ascontiguousarray(
            x[c * M_loc:(c + 1) * M_loc, :].T
        )  # [K, M_loc] f32
        xb_c = xt_c[:KB * P].reshape(KB, P, M_loc).astype(NP_BF16)
        m = {"xb": xb_c, "wb": wb}
        if kf:
            x8_c = (
                xt_c[KB * P:]
                .reshape(kf // 2, 2, P, M_loc)
                .transpose(0, 2, 1, 3)  # [pair, p, slot, m]
                .astype(NP_FP8)
                .reshape(kf // 2, P, 2 * M_loc)
            )
            m["x8"] = np.ascontiguousarray(x8_c)
            m["w8"] = w8
        in_maps.append(m)

    res = bass_utils.run_bass_kernel_spmd(
        nc, in_maps, core_ids=list(range(N_CORES))
    )
    return np.concatenate(
        [
            np.ascontiguousarray(
                res.results[c]["yt"].reshape(O, M_loc).T
            )
            for c in range(N_CORES)
        ],
        axis=0,
    )


# revision 11
# speedup vs baseline: 1.1884x; 1.1884x over previous
"""Self-contained Trainium2 kernel for nn_Linear_14293651161742.

Computes y[m,o] = sum_k x[m,k] * weight[o,k] * w_scale[o//128, k//128]
(the reference's act_quant divide/multiply round-trip is an exact no-op up
to fp32 rounding, far below the matmul noise floor).

Strategy: shard M across the 8 cores (each core reads the full weight once
plus its x slice -- less HBM traffic than the column-parallel hint, which
replicates the much larger x). All scale folding, transposition, and dtype
casts happen on the host; the device runs a pure GEMM stream.

Precision/speed split: the PE runs fp8(e4m3) matmuls at 2x bf16 throughput
via MatmulPerfMode.DoubleRow, but e4m3's 3 mantissa bits give a ~3.7e-2
relative GEMM error -- over the 2e-2 budget. So the K=4096 contraction is
split: 24 of the 32 128-wide k-tiles run in bf16 and 8 run as 4 fp8
DoubleRow pair-tiles (256-deep contraction each, 1 cycle per output
column). Measured end-to-end relative error 1.9e-2; PE cycle count drops
to 28/32 of the all-bf16 kernel.

Schedule: one o-tile (128 outputs) per round; w slabs stream several
rounds ahead on the sync queue while x (fully SBUF-resident) streams on
the scalar queue with the fp8 pairs mid-stream. Round 0 covers four
o-tiles so the PE has enough work per arriving x k-tile to never stall
while x streams in; the first w slabs are loaded in interleaved k-chunks
so every round-0 o-tile's early k-tiles land first. PSUM accumulates per
o-tile in two [128,512] banks (one start, one stop per zero-region), DVE
evicts to SBUF, stores overlap compute.
"""

import sys

if "/opt/trn_rl_repo" not in sys.path:
    sys.path.insert(0, "/opt/trn_rl_repo")

import ml_dtypes
import numpy as np

import concourse.bacc as bacc
import concourse.mybir as mybir
import concourse.tile as tile
from concourse import bass_utils

P = 128
N_CORES = 8
KF_TILES = 8          # k-tiles computed in fp8 (must be even)

F32 = mybir.dt.float32
BF16 = mybir.dt.bfloat16
FP8 = mybir.dt.float8e4

NP_BF16 = ml_dtypes.bfloat16
NP_FP8 = ml_dtypes.float8_e4m3fn


def build_gemm_nc(M_loc: int, K: int, O: int, kf: int):
    """Per-core program: yt[ot, p, m] = sum_k w'[ot*128+p, k] * x[m, k].

    Inputs (per core, all host-prepped):
      xb [KB, P, M_loc]        bf16 : x k-tile kt, partition p = k in tile
      x8 [KF/2, P, 2, M_loc]   fp8  : fp8 k-pair t, slot i -> k tile KB+2t+i
      wb [OB, P, KB*P]         bf16 : per o-tile slab, [p][kt][o] packed
      w8 [OB, P, (KF/2)*2*P]   fp8  : per o-tile slab, [p][t][i][o] packed
    Output:
      yt [OB, P, M_loc]        f32  : y^T slice (host transposes back)
    """
    KT = K // P
    KB = KT - kf           # bf16 k-tiles
    PAIRS = kf // 2        # fp8 DoubleRow pair-tiles
    OB = O // P
    MCW = min(512, M_loc)  # bf16 moving chunk (max 512)
    MC = M_loc // MCW
    DCW = min(256, M_loc)  # fp8 DoubleRow moving chunk (2*256 = 512 max;
                           # wider DR forms run at half rate on hardware)
    DC = M_loc // DCW
    HALVES = MCW // DCW    # DR chunks per psum tile
    DR = mybir.MatmulPerfMode.DoubleRow

    nc = bacc.Bacc("TRN2", target_bir_lowering=False, debug=False)
    xb = nc.dram_tensor("xb", [KB, P, M_loc], BF16, kind="ExternalInput")
    wbt = nc.dram_tensor("wb", [OB, P, KB * P], BF16, kind="ExternalInput")
    yt = nc.dram_tensor("yt", [OB, P, M_loc], F32, kind="ExternalOutput")
    if PAIRS:
        x8 = nc.dram_tensor("x8", [PAIRS, P, 2 * M_loc], FP8, kind="ExternalInput")
        w8t = nc.dram_tensor("w8", [OB, P, PAIRS * 2 * P], FP8, kind="ExternalInput")

    # Round 0 covers four o-tiles so the PE has ~1.7us of work per arriving
    # x k-tile (x streams for ~35us) and never stalls on the x stream;
    # later rounds one o-tile each.
    R0 = min(4, OB)
    rounds = [list(range(R0))] + [[ot] for ot in range(R0, OB)]
    W_PREFETCH = 6         # o-tile w slabs in flight (= pool bufs)

    with tile.TileContext(nc) as tc:
        with (
            tc.tile_pool(name="xpool", bufs=1) as x_pool,
            tc.tile_pool(name="wbp", bufs=W_PREFETCH) as wb_pool,
            tc.tile_pool(name="w8p", bufs=W_PREFETCH) as w8_pool,
            tc.tile_pool(name="yout", bufs=4) as y_pool,
            tc.tile_pool(name="psum", bufs=4, space="PSUM") as psum_pool,
        ):
            wb_sb = {}
            w8_sb = {}

            def alloc_w(ot):
                wb_sb[ot] = wb_pool.tile([P, KB, P], BF16, tag="wb",
                                         name=f"wb{ot}")
                if PAIRS:
                    w8_sb[ot] = w8_pool.tile([P, PAIRS, 2, P], FP8, tag="w8",
                                             name=f"w8{ot}")

            def load_wb_chunk(ot, k0, k1, eng=None):
                (eng or nc.sync).dma_start(
                    wb_sb[ot][:, k0:k1].rearrange("p kt o -> p (kt o)"),
                    wbt.ap()[ot, :, k0 * P:k1 * P],
                )

            def load_w8(ot, eng=None):
                if PAIRS:
                    (eng or nc.sync).dma_start(
                        w8_sb[ot][:].rearrange("p t i o -> p (t i o)"),
                        w8t.ap()[ot],
                    )

            # Prologue. The first two x k-tiles ride at the head of the sync
            # queue (earliest to start) with a tiny first w chunk, so the
            # first matmul fires as soon as the PE sequencer boots; the rest
            # of x streams on the scalar queue (HWDGE), with the fp8 x pairs
            # placed mid-stream so they land well before round 0's DR phase.
            xb_sb = [None] * KB
            x8_sb = [None] * PAIRS

            def load_x(kt, eng):
                t = x_pool.tile([P, M_loc], BF16, tag=f"xb{kt}", name=f"xb{kt}")
                eng.dma_start(t[:], xb.ap()[kt])
                xb_sb[kt] = t

            def load_x8(pr, eng):
                t = x_pool.tile([P, 2, M_loc], FP8, tag=f"x8{pr}", name=f"x8{pr}")
                eng.dma_start(t[:].rearrange("p i m -> p (i m)"), x8.ap()[pr])
                x8_sb[pr] = t

            head = min(2, KB)
            for kt in range(head):
                load_x(kt, nc.sync)

            for ot in range(R0):
                alloc_w(ot)
            bounds = [b for b in (0, 2, KB // 3, (2 * KB) // 3, KB)
                      if 0 <= b <= KB]
            bounds = sorted(set(bounds))
            for g in range(len(bounds) - 1):
                for ot in range(R0):
                    load_wb_chunk(ot, bounds[g], bounds[g + 1])
            for ot in range(R0):
                load_w8(ot)

            mid = KB // 2
            for kt in range(head, mid):
                load_x(kt, nc.scalar)
            for pr in range(PAIRS):
                load_x8(pr, nc.scalar)
            for kt in range(mid, KB):
                load_x(kt, nc.scalar)

            # Prefetch two more slabs behind x on the scalar queue so they
            # don't compete with the round-0 x stream for HBM bandwidth.
            next_w = R0
            for _ in range(min(2, OB - next_w)):
                alloc_w(next_w)
                load_wb_chunk(next_w, 0, KB, nc.scalar)
                load_w8(next_w, nc.scalar)
                next_w += 1

            for rnd in rounds:
                last_round = rnd is rounds[-1]
                psums = {}
                for ot in rnd:
                    for mc in range(MC):
                        psums[(ot, mc)] = psum_pool.tile(
                            [P, MCW], F32, tag=f"ps{mc}", name=f"ps{ot}_{mc}"
                        )
                # bf16 k-tiles (start flag on kt 0, full-width writes)
                for kt in range(KB):
                    for ot in rnd:
                        for mc in range(MC):
                            nc.tensor.matmul(
                                psums[(ot, mc)][:],
                                wb_sb[ot][:, kt],
                                xb_sb[kt][:, mc * MCW:(mc + 1) * MCW],
                                start=(kt == 0),
                                stop=(PAIRS == 0 and kt == KB - 1),
                            )
                # fp8 DoubleRow pair-tiles; stop on the last write per psum
                # tile (zero-region) only. In the last round iterate chunks
                # outermost so psum tile 0 completes (and can evict) while
                # the PE still works on tile 1.
                if PAIRS:
                    dr_order = (
                        [(pr, c) for c in range(DC) for pr in range(PAIRS)]
                        if last_round else
                        [(pr, c) for pr in range(PAIRS) for c in range(DC)]
                    )
                    for ot in rnd:
                        for pr, c in dr_order:
                            mc, half = divmod(c, HALVES)
                            nc.tensor.matmul(
                                psums[(ot, mc)][:, half * DCW:(half + 1) * DCW],
                                w8_sb[ot][:, pr],
                                x8_sb[pr][:, :, c * DCW:(c + 1) * DCW],
                                start=False,
                                stop=(pr == PAIRS - 1 and half == HALVES - 1),
                                perf_mode=DR,
                            )
                # evict on DVE, store on the scalar-engine DMA queue (the
                # last round splits the store across two queues per-mc so
                # the tail drains sooner)
                for ot in rnd:
                    ysb = y_pool.tile([P, M_loc], F32, tag="y", name=f"y{ot}")
                    for mc in range(MC):
                        nc.vector.tensor_copy(
                            ysb[:, mc * MCW:(mc + 1) * MCW], psums[(ot, mc)][:]
                        )
                        if last_round:
                            eng = nc.scalar if mc % 2 == 0 else nc.sync
                            eng.dma_start(
                                yt.ap()[ot, :, mc * MCW:(mc + 1) * MCW],
                                ysb[:, mc * MCW:(mc + 1) * MCW],
                            )
                    if not last_round:
                        nc.gpsimd.dma_start(yt.ap()[ot], ysb[:])
                    del wb_sb[ot]
                    if PAIRS:
                        del w8_sb[ot]
                    if next_w < OB:
                        alloc_w(next_w)
                        load_wb_chunk(next_w, 0, KB)
                        load_w8(next_w)
                        next_w += 1
    nc.compile()
    return nc


_CACHED = {}


def _get_nc(M_loc, K, O, kf):
    key = (M_loc, K, O, kf)
    if key not in _CACHED:
        _CACHED[key] = build_gemm_nc(M_loc, K, O, kf)
    return _CACHED[key]


def _prep_weights(weight: np.ndarray, w_scale: np.ndarray, kf: int):
    O, K = weight.shape
    OB, KT = O // P, K // P
    KB = KT - kf
    wdq = (
        weight.reshape(OB, P, KT, P).astype(np.float32)
        * w_scale[:, None, :, None]
    )  # [ot, o, kt, p]
    # bf16 part: [ot, p(k), kt, o] slabs, contiguous per (ot, p)
    wb = np.ascontiguousarray(
        wdq[:, :, :KB].transpose(0, 3, 2, 1)
    ).astype(NP_BF16).reshape(OB, P, KB * P)
    w8 = None
    if kf:
        # fp8 part: [ot, p(k), pair, slot, o]
        w8f = wdq[:, :, KB:].reshape(OB, P, kf // 2, 2, P)  # [ot,o,t,i,p]
        w8 = np.ascontiguousarray(
            w8f.transpose(0, 4, 2, 3, 1)
        ).astype(NP_FP8).reshape(OB, P, (kf // 2) * 2 * P)
    return wb, w8


def kernel(x: np.ndarray, weight: np.ndarray, w_scale: np.ndarray) -> np.ndarray:
    M, K = x.shape
    O = weight.shape[0]
    assert M % N_CORES == 0
    M_loc = M // N_CORES
    kf = KF_TILES
    KT = K // P
    KB = KT - kf

    nc = _get_nc(M_loc, K, O, kf)
    wb, w8 = _prep_weights(weight, w_scale, kf)

    in_maps = []
    for c in range(N_CORES):
        xt_c = np.ascontiguousarray(
            x[c * M_loc:(c + 1) * M_loc, :].T
        )  # [K, M_loc] f32
        xb_c = xt_c[:KB * P].reshape(KB, P, M_loc).astype(NP_BF16)
        m = {"xb": xb_c, "wb": wb}
        if kf:
            x8_c = (
                xt_c[KB * P:]
                .reshape(kf // 2, 2, P, M_loc)
                .transpose(0, 2, 1, 3)  # [pair, p, slot, m]
                .astype(NP_FP8)
                .reshape(kf // 2, P, 2 * M_loc)
            )
            m["x8"] = np.ascontiguousarray(x8_c)
            m["w8"] = w8
        in_maps.append(m)

    res = bass_utils.run_bass_kernel_spmd(
        nc, in_maps, core_ids=list(range(N_CORES))
    )
    return np.concatenate(
        [
            np.ascontiguousarray(
                res.results[c]["yt"].reshape(O, M_loc).T
            )
            for c in range(N_CORES)
        ],
        axis=0,
    )
